# revision 1
# baseline (speedup 1.0000x reference)
"""GraphConv (DGL norm='both', 3 layers) on 8 trn2 NeuronCores.

Sharding: destination nodes (and their edges) are sharded across the 8
cores. Each layer, a core gathers the source rows of its edge shard from
a replicated node table with dma_gather (1024-row spans, 4 SWDGE
queues), reduces each 128-edge chunk into a per-dst-block [f, d] PSUM
accumulator via a weighted one-hot matmul (weights carry
ns[src]*nd[dst], so the full symmetric normalization folds into the
segment sum), applies W + bias + relu on the dst shard, and AllGathers
the new table for the next layer.

int16 gather indices only address 32768 rows, so the padded node table
(8 shards x 12544 rows = 100352) is split into 4 windows; each dst
block's edges are bucketed by (window) with a per-(block, window) chunk
count fixed across cores at compile time (schedule rebuilt per call
from the actual graph).
"""

import os
import numpy as np

N_NODES = 100000
F_IN = 128
F_HID = 128
F_OUT = 64
NCORES = 8
P = 128
SHARD = N_NODES // NCORES          # 12500 dst nodes per core
BLOCKS = (SHARD + P - 1) // P      # 98 dst blocks of 128
SHARD_PAD = BLOCKS * P             # 12544 rows per shard in the table
TAB_PAD = NCORES * SHARD_PAD       # 100352 rows in the gathered table
WIN = 32768                        # int16 index window
NWIN = (TAB_PAD + WIN - 1) // WIN  # 4
SPAN = 1024                        # max rows per dma_gather

LAST_EXEC_NS = None
LAST_RESULTS = None


def _preprocess(src, dst):
    """Build the static schedule and per-core slot arrays.

    Returns (cbr, gidx, dstl, wgt):
      cbr  [BLOCKS, NWIN] chunk count per (block, window), shared by cores
      gidx [NCORES, 128, nslots//16] int16 wrapped gather indices
      dstl [NCORES, 128, nchunks] f32 local dst (255 for pads)
      wgt  [NCORES, 128, nchunks] f32 ns[src]*nd[dst] (0 for pads)
    Slot streams are window-major: for w in windows, for b in blocks,
    cbr[b, w] chunks of 128 edges.
    """
    src = np.asarray(src).astype(np.int64).ravel()
    dst = np.asarray(dst).astype(np.int64).ravel()
    E = src.shape[0]
    deg_out = np.bincount(src, minlength=N_NODES)
    deg_in = np.bincount(dst, minlength=N_NODES)
    ns = np.power(np.maximum(deg_out, 1.0), -0.5).astype(np.float32)
    nd = np.power(np.maximum(deg_in, 1.0), -0.5).astype(np.float32)

    psrc = (src // SHARD) * SHARD_PAD + (src % SHARD)  # padded table row
    win = psrc // WIN
    core = dst // SHARD
    blk = (dst - core * SHARD) // P

    # sort edges by (core, window, block) to match the stream layout
    key = (core * NWIN + win) * BLOCKS + blk
    order = np.argsort(key, kind="stable")
    s_key = key[order]
    s_psrc = psrc[order]
    s_widx = (s_psrc - win[order] * WIN).astype(np.int16)
    s_loc = dst[order] - core[order] * SHARD
    s_dstl = (s_loc % P).astype(np.float32)
    s_w = (ns[src[order]] * nd[dst[order]]).astype(np.float32)

    counts = np.bincount(s_key, minlength=NCORES * NWIN * BLOCKS)
    counts = counts.reshape(NCORES, NWIN, BLOCKS)
    cbr = np.ceil(counts.max(axis=0) / P).astype(np.int64).T  # [BLOCKS, NWIN]
    cbr = np.maximum(cbr, 1)
    nchunks = int(cbr.sum())
    nslots = nchunks * P

    # slot offset of each (window, block) cell in the stream
    cell_chunk_off = np.zeros((NWIN, BLOCKS), np.int64)
    off = 0
    for w in range(NWIN):
        for b in range(BLOCKS):
            cell_chunk_off[w, b] = off
            off += cbr[b, w]

    starts = np.concatenate([[0], np.cumsum(counts.reshape(-1))[:-1]])
    pos = np.arange(E, dtype=np.int64) - starts[s_key]
    slot = cell_chunk_off[win[order], blk[order]] * P + pos
    # sanity: every edge fits its cell
    assert (pos < cbr[blk[order], win[order]] * P).all()

    gidx_flat = np.zeros((NCORES, nslots), dtype=np.int16)
    dstl_flat = np.full((NCORES, nslots), 255.0, dtype=np.float32)
    wgt_flat = np.zeros((NCORES, nslots), dtype=np.float32)
    flat = core[order] * nslots + slot
    gidx_flat.reshape(-1)[flat] = s_widx
    dstl_flat.reshape(-1)[flat] = s_dstl
    wgt_flat.reshape(-1)[flat] = s_w

    # gather-index wrap: logical idx i -> [i%16 + 16*rep, i//16]
    gw = gidx_flat.reshape(NCORES, nslots // 16, 16)
    gidx = np.ascontiguousarray(np.tile(gw.transpose(0, 2, 1), (1, 8, 1)))
    # chunk-major [p, chunk] layout for dstl/wgt: slot = chunk*128 + p
    def to_pc(a):
        return np.ascontiguousarray(
            a.reshape(NCORES, nchunks, P).transpose(0, 2, 1))

    return cbr, gidx, to_pc(dstl_flat), to_pc(wgt_flat)


def _build_program(cbr):
    import concourse.bacc as bacc
    import concourse.tile as tile
    from concourse import mybir

    f32 = mybir.dt.float32
    i16 = mybir.dt.int16
    AF = mybir.ActivationFunctionType
    ALU = mybir.AluOpType

    nchunks = int(cbr.sum())
    nslots = nchunks * P
    # per-window stream extents (in chunks)
    wlen = [int(cbr[:, w].sum()) for w in range(NWIN)]
    woff = np.concatenate([[0], np.cumsum(wlen)]).astype(int)
    wrows = [min(WIN, TAB_PAD - w * WIN) for w in range(NWIN)]
    # chunk offset of cell (w, b) within the global stream
    cell_off = np.zeros((NWIN, BLOCKS), np.int64)
    off = 0
    for w in range(NWIN):
        for b in range(BLOCKS):
            cell_off[w, b] = off
            off += cbr[b, w]

    nc = bacc.Bacc("TRN2", target_bir_lowering=False, debug=False,
                   num_devices=NCORES, num_swdge_queues=4)

    feats = nc.dram_tensor("featpad", [TAB_PAD, F_IN], f32,
                           kind="ExternalInput")
    w_d = [nc.dram_tensor(f"W{i}", [F_IN, fo], f32, kind="ExternalInput")
           for i, fo in enumerate([F_HID, F_HID, F_OUT])]
    b_d = [nc.dram_tensor(f"b{i}", [fo], f32, kind="ExternalInput")
           for i, fo in enumerate([F_HID, F_HID, F_OUT])]
    gidx_d = nc.dram_tensor("gidx", [P, nslots // 16], i16,
                            kind="ExternalInput")
    dstl_d = nc.dram_tensor("dstl", [P, nchunks], f32, kind="ExternalInput")
    wgt_d = nc.dram_tensor("wgt", [P, nchunks], f32, kind="ExternalInput")
    iota_d = nc.dram_tensor("iota", [P, P], f32, kind="ExternalInput")
    ident_d = nc.dram_tensor("identity", [P, P], f32, kind="ExternalInput")
    out_d = nc.dram_tensor("out", [SHARD, F_OUT], f32, kind="ExternalOutput")

    with tile.TileContext(nc) as tc:
        with (
            tc.tile_pool(name="const", bufs=1) as cpool,
            tc.tile_pool(name="gs", bufs=4) as gpool,
            tc.tile_pool(name="s", bufs=6) as spool,
            tc.tile_pool(name="mid", bufs=3) as mpool,
            tc.tile_pool(name="ps", bufs=2, space="PSUM") as pspool,
            tc.tile_pool(name="dram", bufs=1, space="DRAM") as dpool,
        ):
            iota_sb = cpool.tile([P, P], f32, tag="iota")
            nc.sync.dma_start(out=iota_sb[:], in_=iota_d[:])
            ident_sb = cpool.tile([P, P], f32, tag="ident")
            nc.sync.dma_start(out=ident_sb[:], in_=ident_d[:])
            w_sb, b_sb = [], []
            for i, fo in enumerate([F_HID, F_HID, F_OUT]):
                t = cpool.tile([F_IN, fo], f32, tag=f"w{i}")
                nc.sync.dma_start(out=t[:], in_=w_d[i][:])
                w_sb.append(t)
                t = cpool.tile([fo, 1], f32, tag=f"b{i}")
                nc.sync.dma_start(out=t[:], in_=b_d[i][:, None])
                b_sb.append(t)
            gidx_sb = cpool.tile([P, nslots // 16], i16, tag="gidx")
            nc.sync.dma_start(out=gidx_sb[:], in_=gidx_d[:])
            dstl_sb = cpool.tile([P, nchunks], f32, tag="dstl")
            nc.sync.dma_start(out=dstl_sb[:], in_=dstl_d[:])
            wgt_sb = cpool.tile([P, nchunks], f32, tag="wgt")
            nc.sync.dma_start(out=wgt_sb[:], in_=wgt_d[:])

            ag_in = dpool.tile([SHARD_PAD, F_HID], f32, tag="ag_in")
            hf0 = dpool.tile([TAB_PAD, F_HID], f32, tag="hf0")
            hf1 = dpool.tile([TAB_PAD, F_HID], f32, tag="hf1")
            hf = [hf0, hf1]

            def layer(li, table_ap, fo, relu):
                # spans: per window, chop its chunk stream into <=8-chunk
                # gathers; span_tiles[w] maps span index -> tile
                qn = [0]
                span_tiles = [{} for _ in range(NWIN)]

                def ensure_span(w, s):
                    if s in span_tiles[w]:
                        return
                    c0 = s * (SPAN // P)
                    ck = min(SPAN // P, wlen[w] - c0)
                    rows = ck * P
                    gt = gpool.tile([P, rows], f32, tag=f"g{w}")
                    gcol = (woff[w] + c0) * P // 16
                    nc.gpsimd.dma_gather(
                        out_ap=gt[:].rearrange("p (k f) -> p k f", f=P),
                        in_ap=table_ap[w * WIN: w * WIN + wrows[w], :],
                        idxs_ap=gidx_sb[:, gcol: gcol + rows // 16],
                        num_idxs=rows, num_idxs_reg=rows, elem_size=P,
                        queue_num=qn[0] % 4)
                    qn[0] += 1
                    span_tiles[w][s] = gt

                for b in range(BLOCKS):
                    ps_mt = pspool.tile([P, P], f32, tag="mt")
                    total = int(cbr[b].sum())
                    done = 0
                    for w in range(NWIN):
                        for j in range(int(cbr[b, w])):
                            cg = int(cell_off[w, b]) + j       # global chunk
                            cw = cg - int(woff[w])             # within stream
                            s = cw // (SPAN // P)
                            ensure_span(w, s)
                            gt = span_tiles[w][s]
                            co = cw - s * (SPAN // P)
                            s_t = spool.tile([P, P], f32, tag="s")
                            nc.vector.tensor_scalar(
                                s_t[:], iota_sb[:],
                                dstl_sb[:, cg:cg + 1],
                                wgt_sb[:, cg:cg + 1],
                                ALU.is_equal, ALU.mult)
                            nc.tensor.matmul(
                                ps_mt[:],
                                lhsT=gt[:, co * P:(co + 1) * P],
                                rhs=s_t[:],
                                start=(done == 0), stop=(done == total - 1))
                            done += 1
                    # epilogue: y.T = relu(W.T @ m.T + b); store y
                    mt_sb = mpool.tile([P, P], f32, tag="mt_sb")
                    nc.scalar.activation(mt_sb[:], ps_mt[:], AF.Copy)
                    ps_yt = pspool.tile([fo, P], f32, tag="yt")
                    nc.tensor.matmul(ps_yt[:], lhsT=w_sb[li][:],
                                     rhs=mt_sb[:], start=True, stop=True)
                    yt_sb = mpool.tile([fo, P], f32, tag="yt_sb")
                    nc.scalar.activation(
                        yt_sb[:], ps_yt[:],
                        AF.Relu if relu else AF.Identity, bias=b_sb[li][:])
                    ps_y = pspool.tile([P, fo], f32, tag="y")
                    nc.tensor.transpose(ps_y[:], yt_sb[:],
                                        ident_sb[:fo, :fo])
                    y_sb = mpool.tile([P, fo], f32, tag="y_sb")
                    nc.scalar.activation(y_sb[:], ps_y[:], AF.Copy)
                    if li < 2:
                        nc.sync.dma_start(out=ag_in[b * P:(b + 1) * P, :],
                                          in_=y_sb[:])
                    else:
                        hi = min((b + 1) * P, SHARD)
                        nc.sync.dma_start(out=out_d[b * P:hi, :],
                                          in_=y_sb[:hi - b * P, :])
                if li < 2:
                    nc.gpsimd.collective_compute(
                        "AllGather", mybir.AluOpType.bypass,
                        replica_groups=[list(range(NCORES))],
                        ins=[ag_in.opt()],
                        outs=[hf[li].opt()],
                    )

            layer(0, feats[:], F_HID, True)
            layer(1, hf[0][:], F_HID, True)
            layer(2, hf[1][:], F_OUT, False)

    nc.compile()
    return nc


def kernel(**inputs):
    global LAST_EXEC_NS, LAST_RESULTS
    from concourse.bass_utils import run_bass_kernel_spmd

    cbr, gidx, dstl, wgt = _preprocess(inputs["src"], inputs["dst"])
    nc = _build_program(cbr)

    feats = np.asarray(inputs["features"], dtype=np.float32)
    featpad = np.zeros((TAB_PAD, F_IN), np.float32)
    for c in range(NCORES):
        featpad[c * SHARD_PAD: c * SHARD_PAD + SHARD] = \
            feats[c * SHARD: (c + 1) * SHARD]

    common = {
        "featpad": featpad,
        "W0": np.asarray(inputs["W0"], dtype=np.float32),
        "W1": np.asarray(inputs["W1"], dtype=np.float32),
        "W2": np.asarray(inputs["W2"], dtype=np.float32),
        "b0": np.asarray(inputs["b0"], dtype=np.float32),
        "b1": np.asarray(inputs["b1"], dtype=np.float32),
        "b2": np.asarray(inputs["b2"], dtype=np.float32),
        "iota": np.tile(np.arange(P, dtype=np.float32), (P, 1)),
        "identity": np.eye(P, dtype=np.float32),
    }
    in_maps = []
    for c in range(NCORES):
        m = dict(common)
        m["gidx"] = gidx[c]
        m["dstl"] = dstl[c]
        m["wgt"] = wgt[c]
        in_maps.append(m)

    trace = bool(int(os.environ.get("BASS_GNN_TRACE", "0")))
    kwargs = {}
    if trace:
        _register_ntff_hook()
        kwargs = dict(trace=True,
                      tmpdir=os.environ.get("BASS_GNN_TRACE_DIR") or None)
    res = run_bass_kernel_spmd(nc, in_maps, core_ids=list(range(NCORES)),
                               **kwargs)
    LAST_EXEC_NS = res.exec_time_ns
    LAST_RESULTS = res
    out = np.concatenate([res.results[c]["out"] for c in range(NCORES)],
                         axis=0)
    return np.ascontiguousarray(out.astype(np.float32))


def _register_ntff_hook():
    """The container's antenv lacks axon_hooks; register the NTFF profile
    hook ourselves so trace=True works under axon."""
    import sys, types
    if "antenv.axon_hooks" in sys.modules:
        return
    try:
        import antenv
        from trn_agent_boot.trn_boot import _ntff_profile_via_ctypes
        mod = types.ModuleType("antenv.axon_hooks")
        mod._hook = _ntff_profile_via_ctypes('/opt/axon/libaxon_pjrt.so')
        mod.set_axon_ntff_profile_hook = lambda h: setattr(mod, "_hook", h)
        mod.get_axon_ntff_profile_hook = lambda: mod._hook
        sys.modules["antenv.axon_hooks"] = mod
        antenv.axon_hooks = mod
    except Exception as e:
        print("ntff hook registration failed:", e)



# revision 8
# speedup vs baseline: 1.6712x; 1.6712x over previous
"""GraphConv (DGL norm='both', 3 layers) on 8 trn2 NeuronCores.

Transform-first fp16 datapath. Destination nodes (and their edges) are
sharded across the 8 cores. The replicated node table for layer l holds
t_l = h_l @ W_l in fp16 (layer 0 gathers raw fp16 features instead and
applies W0 in the epilogue). Per dst block a core reduces 128-edge
chunks into a PSUM accumulator via a weighted one-hot matmul (weights
carry ns[src]*nd[dst], so the full symmetric normalization folds into
the segment sum), applies bias+relu plus the NEXT layer's weight matmul
(f32), transposes, and writes the new table shard, which is AllGathered
in two halves so the first half overlaps the tail of the layer.

int16 gather indices only address 32768 rows, so the padded node table
(100352 rows) is split into 4 windows. Table rows are laid out
half-major (all cores' first 6272 rows, then all cores' second halves)
so each half-AllGather writes a contiguous range. Per-(block, window)
chunk counts are fixed across cores at compile time (schedule rebuilt
per call from the actual graph).
"""

import os
import numpy as np

N_NODES = 100000
F_IN = 128
F_HID = 128
F_OUT = 64
NCORES = 8
P = 128
SHARD = N_NODES // NCORES          # 12500 dst nodes per core
BLOCKS = (SHARD + P - 1) // P      # 98 dst blocks of 128
SHARD_PAD = BLOCKS * P             # 12544 rows per shard in the table
HALF = SHARD_PAD // 2              # 6272 rows (49 blocks) per AG half
TAB_PAD = NCORES * SHARD_PAD       # 100352 rows in the gathered table
WIN = 32768                        # int16 index window (max rows)
HALFTAB = NCORES * HALF            # 50176 rows per half table
# window boundaries: <=32768 rows each, aligned to the half boundary so
# each window lives entirely in one of the two Shared half-tables
WBOUND = [0, WIN, HALFTAB, HALFTAB + WIN, TAB_PAD]
NWIN = len(WBOUND) - 1             # 4
SPAN = int(os.environ.get("BASS_GNN_SPAN", "4096"))  # max rows per dma_gather
SPANC = SPAN // P                  # chunks per span
SCRATCH = int(os.environ.get("BASS_GNN_SCRATCH", "65536"))
SHARED_TAB = os.environ.get("BASS_GNN_SHARED", "1") == "1"
ADDR = "Shared" if SHARED_TAB else "Local"

LAST_EXEC_NS = None
LAST_RESULTS = None


def _tab_row(node):
    """Node id -> padded table row (half-major layout for split AG)."""
    c = node // SHARD
    r = node % SHARD
    return np.where(r < HALF,
                    c * HALF + r,
                    NCORES * HALF + c * HALF + (r - HALF))


def _preprocess(src, dst):
    """Build the static schedule and per-core slot arrays.

    Returns (cbr, gidx, dstl, wgt):
      cbr  [BLOCKS, NWIN] chunk count per (block, window), shared by cores
      gidx [NCORES, 128, nslots//16] int16 wrapped gather indices
      dstl [NCORES, 128, nchunks] f32 local dst (255 for pads)
      wgt  [NCORES, 128, nchunks] f32 ns[src]*nd[dst] (0 for pads)
    Slot streams are window-major: for w in windows, for b in blocks,
    cbr[b, w] chunks of 128 edges.
    """
    src = np.asarray(src).astype(np.int64).ravel()
    dst = np.asarray(dst).astype(np.int64).ravel()
    E = src.shape[0]
    deg_out = np.bincount(src, minlength=N_NODES)
    deg_in = np.bincount(dst, minlength=N_NODES)
    ns = np.power(np.maximum(deg_out, 1.0), -0.5).astype(np.float32)
    nd = np.power(np.maximum(deg_in, 1.0), -0.5).astype(np.float32)

    psrc = _tab_row(src)
    win = np.searchsorted(WBOUND, psrc, side="right") - 1
    core = dst // SHARD
    blk = (dst - core * SHARD) // P

    # sort edges by (core, window, block) to match the stream layout
    key = (core * NWIN + win) * BLOCKS + blk
    order = np.argsort(key, kind="stable")
    s_key = key[order]
    s_psrc = psrc[order]
    s_widx = (s_psrc - np.asarray(WBOUND)[win[order]]).astype(np.int16)
    s_loc = dst[order] - core[order] * SHARD
    s_dstl = (s_loc % P).astype(np.float32)
    s_w = (ns[src[order]] * nd[dst[order]]).astype(np.float32)

    counts = np.bincount(s_key, minlength=NCORES * NWIN * BLOCKS)
    counts = counts.reshape(NCORES, NWIN, BLOCKS)
    cbr = np.ceil(counts.max(axis=0) / P).astype(np.int64).T  # [BLOCKS, NWIN]
    cbr = np.maximum(cbr, 1)
    nchunks = int(cbr.sum())
    nslots = nchunks * P

    # slot offset of each (window, block) cell in the stream
    cell_chunk_off = np.zeros((NWIN, BLOCKS), np.int64)
    off = 0
    for w in range(NWIN):
        for b in range(BLOCKS):
            cell_chunk_off[w, b] = off
            off += cbr[b, w]

    starts = np.concatenate([[0], np.cumsum(counts.reshape(-1))[:-1]])
    pos = np.arange(E, dtype=np.int64) - starts[s_key]
    slot = cell_chunk_off[win[order], blk[order]] * P + pos
    # sanity: every edge fits its cell
    assert (pos < cbr[blk[order], win[order]] * P).all()

    gidx_flat = np.zeros((NCORES, nslots), dtype=np.int16)
    dstl_flat = np.full((NCORES, nslots), 255.0, dtype=np.float32)
    wgt_flat = np.zeros((NCORES, nslots), dtype=np.float32)
    flat = core[order] * nslots + slot
    gidx_flat.reshape(-1)[flat] = s_widx
    dstl_flat.reshape(-1)[flat] = s_dstl
    wgt_flat.reshape(-1)[flat] = s_w

    # gather-index wrap: logical idx i -> [i%16 + 16*rep, i//16]
    gw = gidx_flat.reshape(NCORES, nslots // 16, 16)
    gidx = np.ascontiguousarray(np.tile(gw.transpose(0, 2, 1), (1, 8, 1)))
    # chunk-major [p, chunk] layout for dstl/wgt: slot = chunk*128 + p
    def to_pc(a):
        return np.ascontiguousarray(
            a.reshape(NCORES, nchunks, P).transpose(0, 2, 1))

    return cbr, gidx, to_pc(dstl_flat), to_pc(wgt_flat)


def _build_program(cbr):
    import concourse.bacc as bacc
    import concourse.tile as tile
    from concourse import mybir

    f32 = mybir.dt.float32
    f16 = mybir.dt.float16
    i16 = mybir.dt.int16
    AF = mybir.ActivationFunctionType
    ALU = mybir.AluOpType

    nchunks = int(cbr.sum())
    nslots = nchunks * P
    # per-window stream extents (in chunks)
    wlen = [int(cbr[:, w].sum()) for w in range(NWIN)]
    woff = np.concatenate([[0], np.cumsum(wlen)]).astype(int)
    wrows = [WBOUND[w + 1] - WBOUND[w] for w in range(NWIN)]
    # chunk offset of cell (w, b) within the global stream
    cell_off = np.zeros((NWIN, BLOCKS), np.int64)
    off = 0
    for w in range(NWIN):
        for b in range(BLOCKS):
            cell_off[w, b] = off
            off += cbr[b, w]

    nc = bacc.Bacc("TRN2", target_bir_lowering=False, debug=False,
                   num_devices=NCORES, num_swdge_queues=4,
                   dynamic_dma_scratch_size=SCRATCH)

    feats = nc.dram_tensor("featpad", [TAB_PAD, F_IN], f16,
                           kind="ExternalInput")
    w_d = [nc.dram_tensor(f"W{i}", [F_IN, fo], f32, kind="ExternalInput")
           for i, fo in enumerate([F_HID, F_HID, F_OUT])]
    b_d = [nc.dram_tensor(f"b{i}", [fo], f32, kind="ExternalInput")
           for i, fo in enumerate([F_HID, F_HID, F_OUT])]
    gidx_d = nc.dram_tensor("gidx", [P, nslots // 16], i16,
                            kind="ExternalInput")
    dstl_d = nc.dram_tensor("dstl", [P, nchunks], f32, kind="ExternalInput")
    wgt_d = nc.dram_tensor("wgt", [P, nchunks], f32, kind="ExternalInput")
    iota_d = nc.dram_tensor("iota", [P, P], f16, kind="ExternalInput")
    ident_d = nc.dram_tensor("identity", [P, P], f16, kind="ExternalInput")
    identf_d = nc.dram_tensor("identityf", [P, P], f32, kind="ExternalInput")
    out_d = nc.dram_tensor("out", [SHARD, F_OUT], f32, kind="ExternalOutput")

    with tile.TileContext(nc) as tc:
        with (
            tc.tile_pool(name="const", bufs=1) as cpool,
            tc.tile_pool(name="gs", bufs=2) as gpool,
            tc.tile_pool(name="s", bufs=8) as spool,
            tc.tile_pool(name="mid", bufs=3) as mpool,
            tc.tile_pool(name="ps", bufs=2, space="PSUM") as pspool,
            tc.tile_pool(name="dram", bufs=1, space="DRAM") as dpool,
        ):
            iota_sb = cpool.tile([P, P], f16, tag="iota")
            nc.sync.dma_start(out=iota_sb[:], in_=iota_d[:])
            ident_sb = cpool.tile([P, P], f16, tag="ident")
            nc.sync.dma_start(out=ident_sb[:], in_=ident_d[:])
            identf_sb = cpool.tile([P, P], f32, tag="identf")
            nc.sync.dma_start(out=identf_sb[:], in_=identf_d[:])
            w_sb, b_sb = [], []
            for i, fo in enumerate([F_HID, F_HID, F_OUT]):
                t = cpool.tile([F_IN, fo], f32, tag=f"w{i}")
                nc.sync.dma_start(out=t[:], in_=w_d[i][:])
                w_sb.append(t)
                t = cpool.tile([fo, 1], f32, tag=f"b{i}")
                nc.sync.dma_start(out=t[:], in_=b_d[i][:, None])
                b_sb.append(t)
            gidx_sb = cpool.tile([P, nslots // 16], i16, tag="gidx")
            nc.sync.dma_start(out=gidx_sb[:], in_=gidx_d[:])
            dstl_sb = cpool.tile([P, nchunks], f32, tag="dstl")
            nc.sync.dma_start(out=dstl_sb[:], in_=dstl_d[:])
            wgt_sb = cpool.tile([P, nchunks], f32, tag="wgt")
            nc.sync.dma_start(out=wgt_sb[:], in_=wgt_d[:])

            ag1 = dpool.tile([SHARD_PAD, F_HID], f16, tag="ag1")
            ag2 = dpool.tile([SHARD_PAD, P], f16, tag="ag2")
            t1_ta = dpool.tile([HALFTAB, F_HID], f16, tag="t1a",
                               addr_space=ADDR)
            t1_tb = dpool.tile([HALFTAB, F_HID], f16, tag="t1b",
                               addr_space=ADDR)
            t2_ta = dpool.tile([HALFTAB, P], f16, tag="t2a",
                               addr_space=ADDR)
            t2_tb = dpool.tile([HALFTAB, P], f16, tag="t2b",
                               addr_space=ADDR)

            def layer(li, table_slice, gw, relu):
                """gw: gathered row width (elems of fp16 table row)."""
                qn = [0]
                span_tiles = [{} for _ in range(NWIN)]

                def ensure_span(w, s):
                    if s in span_tiles[w]:
                        return
                    c0 = s * SPANC
                    ck = min(SPANC, wlen[w] - c0)
                    rows = ck * P
                    gt = gpool.tile([P, SPANC * gw], f16, tag=f"g{w}")
                    gcol = (woff[w] + c0) * P // 16
                    nc.gpsimd.dma_gather(
                        out_ap=gt[:, :ck * gw].rearrange(
                            "p (k f) -> p k f", f=gw),
                        in_ap=table_slice(w),
                        idxs_ap=gidx_sb[:, gcol: gcol + rows // 16],
                        num_idxs=rows, num_idxs_reg=rows, elem_size=gw,
                        queue_num=qn[0] % 4)
                    qn[0] += 1
                    span_tiles[w][s] = gt

                for b in range(BLOCKS):
                    fa = F_HID if li == 0 else (F_HID if li == 1 else F_OUT)
                    ps_agg = pspool.tile([fa, P], f32, tag="agg")
                    total = int(cbr[b].sum())
                    done = 0
                    for w in range(NWIN):
                        for j in range(int(cbr[b, w])):
                            cg = int(cell_off[w, b]) + j       # global chunk
                            cw = cg - int(woff[w])             # within stream
                            s = cw // SPANC
                            ensure_span(w, s)
                            gt = span_tiles[w][s]
                            co = cw - s * SPANC
                            s_t = spool.tile([P, P], f16, tag="s")
                            nc.vector.tensor_scalar(
                                s_t[:], iota_sb[:],
                                dstl_sb[:, cg:cg + 1],
                                wgt_sb[:, cg:cg + 1],
                                ALU.is_equal, ALU.mult)
                            nc.tensor.matmul(
                                ps_agg[:],
                                lhsT=gt[:, co * gw: co * gw + fa],
                                rhs=s_t[:],
                                start=(done == 0), stop=(done == total - 1))
                            done += 1
                    # epilogue
                    if li == 0:
                        # psum = (sum_e w_e feats[src_e])^T [feat, dst]
                        mt_sb = mpool.tile([P, P], f32, tag="mt")
                        nc.scalar.activation(mt_sb[:], ps_agg[:], AF.Copy)
                        ps_h = pspool.tile([F_HID, P], f32, tag="ph")
                        nc.tensor.matmul(ps_h[:], lhsT=w_sb[0][:],
                                         rhs=mt_sb[:], start=True, stop=True)
                        h_sb = mpool.tile([F_HID, P], f32, tag="h")
                        nc.scalar.activation(h_sb[:], ps_h[:], AF.Relu,
                                             bias=b_sb[0][:])
                        ps_t = pspool.tile([F_HID, P], f32, tag="pt")
                        nc.tensor.matmul(ps_t[:], lhsT=w_sb[1][:],
                                         rhs=h_sb[:], start=True, stop=True)
                        tt_sb = mpool.tile([F_HID, P], f16, tag="tt")
                        nc.scalar.activation(tt_sb[:], ps_t[:], AF.Copy)
                        ps_tr = pspool.tile([P, F_HID], f16, tag="ptr")
                        nc.tensor.transpose(ps_tr[:], tt_sb[:], ident_sb[:])
                        y_sb = mpool.tile([P, F_HID], f16, tag="y")
                        nc.vector.tensor_copy(y_sb[:], ps_tr[:])
                        nc.sync.dma_start(
                            out=ag1[b * P:(b + 1) * P, :], in_=y_sb[:])
                    elif li == 1:
                        # psum = (agg*nd)^T [f1', dst]; produce t2 block
                        h_sb = mpool.tile([F_HID, P], f32, tag="h")
                        nc.scalar.activation(h_sb[:], ps_agg[:], AF.Relu,
                                             bias=b_sb[1][:])
                        ps_t = pspool.tile([F_OUT, P], f32, tag="pt")
                        nc.tensor.matmul(ps_t[:], lhsT=w_sb[2][:],
                                         rhs=h_sb[:], start=True, stop=True)
                        tt_sb = mpool.tile([F_OUT, P], f16, tag="tt")
                        nc.scalar.activation(tt_sb[:], ps_t[:], AF.Copy)
                        ps_tr = pspool.tile([P, F_OUT], f16, tag="ptr")
                        nc.tensor.transpose(ps_tr[:], tt_sb[:],
                                            ident_sb[:F_OUT, :F_OUT])
                        y_sb = mpool.tile([P, F_OUT], f16, tag="y")
                        nc.vector.tensor_copy(y_sb[:], ps_tr[:])
                        nc.sync.dma_start(
                            out=ag2[b * P:(b + 1) * P, :F_OUT], in_=y_sb[:])
                    else:
                        # psum = (agg*nd)^T [64, dst]; final output
                        yt_sb = mpool.tile([F_OUT, P], f32, tag="yt")
                        nc.scalar.activation(yt_sb[:], ps_agg[:], AF.Identity,
                                             bias=b_sb[2][:])
                        ps_tr = pspool.tile([P, F_OUT], f32, tag="ptr")
                        nc.tensor.transpose(ps_tr[:], yt_sb[:],
                                            identf_sb[:F_OUT, :F_OUT])
                        y_sb = mpool.tile([P, F_OUT], f32, tag="y")
                        nc.vector.tensor_copy(y_sb[:], ps_tr[:])
                        hi = min((b + 1) * P, SHARD)
                        nc.sync.dma_start(out=out_d[b * P:hi, :],
                                          in_=y_sb[:hi - b * P, :])
                    # split AllGather: first half as soon as block 48 done
                    if li < 2 and b == BLOCKS // 2 - 1:
                        ag_t = ag1 if li == 0 else ag2
                        tab = t1_ta if li == 0 else t2_ta
                        nc.gpsimd.collective_compute(
                            "AllGather", mybir.AluOpType.bypass,
                            replica_groups=[list(range(NCORES))],
                            ins=[ag_t[0:HALF, :].opt()],
                            outs=[tab[:].opt()],
                        )
                if li < 2:
                    ag_t = ag1 if li == 0 else ag2
                    tab = t1_tb if li == 0 else t2_tb
                    nc.gpsimd.collective_compute(
                        "AllGather", mybir.AluOpType.bypass,
                        replica_groups=[list(range(NCORES))],
                        ins=[ag_t[HALF:SHARD_PAD, :].opt()],
                        outs=[tab[:].opt()],
                    )

            def _tslice(ta, tb):
                def f(w):
                    if WBOUND[w] < HALFTAB:
                        return ta[WBOUND[w]:WBOUND[w + 1], :]
                    return tb[WBOUND[w] - HALFTAB:WBOUND[w + 1] - HALFTAB, :]
                return f

            layer(0, lambda w: feats[WBOUND[w]:WBOUND[w + 1], :], F_IN, True)
            layer(1, _tslice(t1_ta, t1_tb), F_HID, True)
            layer(2, _tslice(t2_ta, t2_tb), P, False)

    nc.compile()
    return nc


def kernel(**inputs):
    global LAST_EXEC_NS, LAST_RESULTS
    from concourse.bass_utils import run_bass_kernel_spmd

    cbr, gidx, dstl, wgt = _preprocess(inputs["src"], inputs["dst"])
    nc = _build_program(cbr)

    feats = np.asarray(inputs["features"], dtype=np.float32)
    featpad = np.zeros((TAB_PAD, F_IN), np.float16)
    rows = _tab_row(np.arange(N_NODES))
    featpad[rows] = feats.astype(np.float16)

    common = {
        "featpad": featpad,
        "W0": np.asarray(inputs["W0"], dtype=np.float32),
        "W1": np.asarray(inputs["W1"], dtype=np.float32),
        "W2": np.asarray(inputs["W2"], dtype=np.float32),
        "b0": np.asarray(inputs["b0"], dtype=np.float32),
        "b1": np.asarray(inputs["b1"], dtype=np.float32),
        "b2": np.asarray(inputs["b2"], dtype=np.float32),
        "iota": np.tile(np.arange(P, dtype=np.float16), (P, 1)),
        "identity": np.eye(P, dtype=np.float16),
        "identityf": np.eye(P, dtype=np.float32),
    }
    in_maps = []
    for c in range(NCORES):
        m = dict(common)
        m["gidx"] = gidx[c]
        m["dstl"] = dstl[c]
        m["wgt"] = wgt[c]
        in_maps.append(m)

    trace = bool(int(os.environ.get("BASS_GNN_TRACE", "0")))
    kwargs = {}
    if trace:
        _register_ntff_hook()
        kwargs = dict(trace=True,
                      tmpdir=os.environ.get("BASS_GNN_TRACE_DIR") or None)
    res = run_bass_kernel_spmd(nc, in_maps, core_ids=list(range(NCORES)),
                               **kwargs)
    LAST_EXEC_NS = res.exec_time_ns
    LAST_RESULTS = res
    out = np.concatenate([res.results[c]["out"] for c in range(NCORES)],
                         axis=0)
    return np.ascontiguousarray(out.astype(np.float32))


def _register_ntff_hook():
    """The container's antenv lacks axon_hooks; register the NTFF profile
    hook ourselves so trace=True works under axon."""
    import sys, types
    if "antenv.axon_hooks" in sys.modules:
        return
    try:
        import antenv
        from trn_agent_boot.trn_boot import _ntff_profile_via_ctypes
        mod = types.ModuleType("antenv.axon_hooks")
        mod._hook = _ntff_profile_via_ctypes('/opt/axon/libaxon_pjrt.so')
        mod.set_axon_ntff_profile_hook = lambda h: setattr(mod, "_hook", h)
        mod.get_axon_ntff_profile_hook = lambda: mod._hook
        sys.modules["antenv.axon_hooks"] = mod
        antenv.axon_hooks = mod
    except Exception as e:
        print("ntff hook registration failed:", e)


# revision 10
# speedup vs baseline: 1.7873x; 1.0695x over previous
"""GraphConv (DGL norm='both', 3 layers) on 8 trn2 NeuronCores.

Transform-first fp16 datapath. Destination nodes (and their edges) are
sharded across the 8 cores. The replicated node table for layer l holds
t_l = h_l @ W_l in fp16 (layer 0 gathers raw fp16 features instead and
applies W0 in the epilogue). Per dst block a core reduces 128-edge
chunks into a PSUM accumulator via a weighted one-hot matmul (weights
carry ns[src]*nd[dst], so the full symmetric normalization folds into
the segment sum), applies bias+relu plus the NEXT layer's weight matmul
(f32), transposes, and writes the new table shard, which is AllGathered
in two halves so the first half overlaps the tail of the layer.

int16 gather indices only address 32768 rows, so the padded node table
(100352 rows) is split into 4 windows. Table rows are laid out
half-major (all cores' first 6272 rows, then all cores' second halves)
so each half-AllGather writes a contiguous range. Per-(block, window)
chunk counts are fixed across cores at compile time (schedule rebuilt
per call from the actual graph).
"""

import os
import numpy as np

N_NODES = 100000
F_IN = 128
F_HID = 128
F_OUT = 64
NCORES = 8
P = 128
SHARD = N_NODES // NCORES          # 12500 dst nodes per core
BLOCKS = (SHARD + P - 1) // P      # 98 dst blocks of 128
SHARD_PAD = BLOCKS * P             # 12544 rows per shard in the table
HALF = SHARD_PAD // 2              # 6272 rows (49 blocks) per AG half
TAB_PAD = NCORES * SHARD_PAD       # 100352 rows in the gathered table
WIN = 32768                        # int16 index window (max rows)
HALFTAB = NCORES * HALF            # 50176 rows per half table
# window boundaries: <=32768 rows each, aligned to the half boundary so
# each window lives entirely in one of the two Shared half-tables
WBOUND = [0, WIN, HALFTAB, HALFTAB + WIN, TAB_PAD]
NWIN = len(WBOUND) - 1             # 4
SPAN = int(os.environ.get("BASS_GNN_SPAN", "1024"))  # max rows per dma_gather
SPANC = SPAN // P                  # chunks per span
SCRATCH = int(os.environ.get("BASS_GNN_SCRATCH", "16384"))
SHARED_TAB = os.environ.get("BASS_GNN_SHARED", "1") == "1"
ADDR = "Shared" if SHARED_TAB else "Local"

LAST_EXEC_NS = None
LAST_RESULTS = None


def _tab_row(node):
    """Node id -> padded table row (half-major layout for split AG)."""
    c = node // SHARD
    r = node % SHARD
    return np.where(r < HALF,
                    c * HALF + r,
                    NCORES * HALF + c * HALF + (r - HALF))


def _preprocess(src, dst):
    """Build the static schedule and per-core slot arrays.

    Returns (cbr, gidx, dstl, wgt):
      cbr  [BLOCKS, NWIN] chunk count per (block, window), shared by cores
      gidx [NCORES, 128, nslots//16] int16 wrapped gather indices
      dstl [NCORES, 128, nchunks] f32 local dst (255 for pads)
      wgt  [NCORES, 128, nchunks] f32 ns[src]*nd[dst] (0 for pads)
    Slot streams are window-major: for w in windows, for b in blocks,
    cbr[b, w] chunks of 128 edges.
    """
    src = np.asarray(src).astype(np.int64).ravel()
    dst = np.asarray(dst).astype(np.int64).ravel()
    E = src.shape[0]
    deg_out = np.bincount(src, minlength=N_NODES)
    deg_in = np.bincount(dst, minlength=N_NODES)
    ns = np.power(np.maximum(deg_out, 1.0), -0.5).astype(np.float32)
    nd = np.power(np.maximum(deg_in, 1.0), -0.5).astype(np.float32)

    psrc = _tab_row(src)
    win = np.searchsorted(WBOUND, psrc, side="right") - 1
    core = dst // SHARD
    blk = (dst - core * SHARD) // P

    # sort edges by (core, window, block) to match the stream layout
    key = (core * NWIN + win) * BLOCKS + blk
    order = np.argsort(key, kind="stable")
    s_key = key[order]
    s_psrc = psrc[order]
    s_widx = (s_psrc - np.asarray(WBOUND)[win[order]]).astype(np.int16)
    s_loc = dst[order] - core[order] * SHARD
    s_dstl = (s_loc % P).astype(np.float32)
    s_w = (ns[src[order]] * nd[dst[order]]).astype(np.float32)

    counts = np.bincount(s_key, minlength=NCORES * NWIN * BLOCKS)
    counts = counts.reshape(NCORES, NWIN, BLOCKS)
    cbr = np.ceil(counts.max(axis=0) / P).astype(np.int64).T  # [BLOCKS, NWIN]
    cbr = np.maximum(cbr, 1)
    nchunks = int(cbr.sum())
    nslots = nchunks * P

    # slot offset of each (window, block) cell in the stream
    cell_chunk_off = np.zeros((NWIN, BLOCKS), np.int64)
    off = 0
    for w in range(NWIN):
        for b in range(BLOCKS):
            cell_chunk_off[w, b] = off
            off += cbr[b, w]

    starts = np.concatenate([[0], np.cumsum(counts.reshape(-1))[:-1]])
    pos = np.arange(E, dtype=np.int64) - starts[s_key]
    slot = cell_chunk_off[win[order], blk[order]] * P + pos
    # sanity: every edge fits its cell
    assert (pos < cbr[blk[order], win[order]] * P).all()

    gidx_flat = np.zeros((NCORES, nslots), dtype=np.int16)
    dstl_flat = np.full((NCORES, nslots), 255.0, dtype=np.float32)
    wgt_flat = np.zeros((NCORES, nslots), dtype=np.float32)
    flat = core[order] * nslots + slot
    gidx_flat.reshape(-1)[flat] = s_widx
    dstl_flat.reshape(-1)[flat] = s_dstl
    wgt_flat.reshape(-1)[flat] = s_w

    # gather-index wrap: logical idx i -> [i%16 + 16*rep, i//16]
    gw = gidx_flat.reshape(NCORES, nslots // 16, 16)
    gidx = np.ascontiguousarray(np.tile(gw.transpose(0, 2, 1), (1, 8, 1)))
    # chunk-major [p, chunk] layout for dstl/wgt: slot = chunk*128 + p
    def to_pc(a):
        return np.ascontiguousarray(
            a.reshape(NCORES, nchunks, P).transpose(0, 2, 1))

    return cbr, gidx, to_pc(dstl_flat), to_pc(wgt_flat)


def _build_program(cbr):
    import concourse.bacc as bacc
    import concourse.tile as tile
    from concourse import mybir

    f32 = mybir.dt.float32
    f16 = mybir.dt.float16
    i16 = mybir.dt.int16
    AF = mybir.ActivationFunctionType
    ALU = mybir.AluOpType

    nchunks = int(cbr.sum())
    nslots = nchunks * P
    # per-window stream extents (in chunks)
    wlen = [int(cbr[:, w].sum()) for w in range(NWIN)]
    woff = np.concatenate([[0], np.cumsum(wlen)]).astype(int)
    wrows = [WBOUND[w + 1] - WBOUND[w] for w in range(NWIN)]
    # chunk offset of cell (w, b) within the global stream
    cell_off = np.zeros((NWIN, BLOCKS), np.int64)
    off = 0
    for w in range(NWIN):
        for b in range(BLOCKS):
            cell_off[w, b] = off
            off += cbr[b, w]

    nc = bacc.Bacc("TRN2", target_bir_lowering=False, debug=False,
                   num_devices=NCORES, num_swdge_queues=4,
                   dynamic_dma_scratch_size=SCRATCH)

    feats = nc.dram_tensor("featpad", [TAB_PAD, F_IN], f16,
                           kind="ExternalInput")
    w_d = [nc.dram_tensor(f"W{i}", [F_IN, fo], f32, kind="ExternalInput")
           for i, fo in enumerate([F_HID, F_HID, F_OUT])]
    b_d = [nc.dram_tensor(f"b{i}", [fo], f32, kind="ExternalInput")
           for i, fo in enumerate([F_HID, F_HID, F_OUT])]
    gidx_d = nc.dram_tensor("gidx", [P, nslots // 16], i16,
                            kind="ExternalInput")
    dstl_d = nc.dram_tensor("dstl", [P, nchunks], f32, kind="ExternalInput")
    wgt_d = nc.dram_tensor("wgt", [P, nchunks], f32, kind="ExternalInput")
    iota_d = nc.dram_tensor("iota", [P, P], f16, kind="ExternalInput")
    ident_d = nc.dram_tensor("identity", [P, P], f16, kind="ExternalInput")
    identf_d = nc.dram_tensor("identityf", [P, P], f32, kind="ExternalInput")
    out_d = nc.dram_tensor("out", [SHARD, F_OUT], f32, kind="ExternalOutput")

    with tile.TileContext(nc) as tc:
        with (
            tc.tile_pool(name="const", bufs=1) as cpool,
            tc.tile_pool(name="gs", bufs=6) as gpool,
            tc.tile_pool(name="s", bufs=16) as spool,
            tc.tile_pool(name="mid", bufs=3) as mpool,
            tc.tile_pool(name="ps", bufs=2, space="PSUM") as pspool,
            tc.tile_pool(name="dram", bufs=1, space="DRAM") as dpool,
        ):
            iota_sb = cpool.tile([P, P], f16, tag="iota")
            nc.sync.dma_start(out=iota_sb[:], in_=iota_d[:])
            ident_sb = cpool.tile([P, P], f16, tag="ident")
            nc.sync.dma_start(out=ident_sb[:], in_=ident_d[:])
            identf_sb = cpool.tile([P, P], f32, tag="identf")
            nc.sync.dma_start(out=identf_sb[:], in_=identf_d[:])
            w_sb, b_sb = [], []
            for i, fo in enumerate([F_HID, F_HID, F_OUT]):
                t = cpool.tile([F_IN, fo], f32, tag=f"w{i}")
                nc.sync.dma_start(out=t[:], in_=w_d[i][:])
                w_sb.append(t)
                t = cpool.tile([fo, 1], f32, tag=f"b{i}")
                nc.sync.dma_start(out=t[:], in_=b_d[i][:, None])
                b_sb.append(t)
            gidx_sb = cpool.tile([P, nslots // 16], i16, tag="gidx")
            nc.sync.dma_start(out=gidx_sb[:], in_=gidx_d[:])
            dstl_sb = cpool.tile([P, nchunks], f32, tag="dstl")
            nc.sync.dma_start(out=dstl_sb[:], in_=dstl_d[:])
            wgt_sb = cpool.tile([P, nchunks], f32, tag="wgt")
            nc.sync.dma_start(out=wgt_sb[:], in_=wgt_d[:])

            ag1 = dpool.tile([SHARD_PAD, F_HID], f16, tag="ag1")
            ag2 = dpool.tile([SHARD_PAD, P], f16, tag="ag2")
            t1_ta = dpool.tile([HALFTAB, F_HID], f16, tag="t1a",
                               addr_space=ADDR)
            t1_tb = dpool.tile([HALFTAB, F_HID], f16, tag="t1b",
                               addr_space=ADDR)
            t2_ta = dpool.tile([HALFTAB, P], f16, tag="t2a",
                               addr_space=ADDR)
            t2_tb = dpool.tile([HALFTAB, P], f16, tag="t2b",
                               addr_space=ADDR)

            def layer(li, table_slice, gw, relu):
                """gw: gathered row width (elems of fp16 table row)."""
                qn = [0]
                span_tiles = [{} for _ in range(NWIN)]

                def ensure_span(w, s):
                    if s in span_tiles[w]:
                        return
                    c0 = s * SPANC
                    ck = min(SPANC, wlen[w] - c0)
                    rows = ck * P
                    gt = gpool.tile([P, SPANC * gw], f16, tag=f"g{w}")
                    gcol = (woff[w] + c0) * P // 16
                    nc.gpsimd.dma_gather(
                        out_ap=gt[:, :ck * gw].rearrange(
                            "p (k f) -> p k f", f=gw),
                        in_ap=table_slice(w),
                        idxs_ap=gidx_sb[:, gcol: gcol + rows // 16],
                        num_idxs=rows, num_idxs_reg=rows, elem_size=gw,
                        queue_num=qn[0] % 4)
                    qn[0] += 1
                    span_tiles[w][s] = gt

                for b in range(BLOCKS):
                    fa = F_HID if li == 0 else (F_HID if li == 1 else F_OUT)
                    ps_agg = pspool.tile([fa, P], f32, tag="agg")
                    total = int(cbr[b].sum())
                    done = 0
                    for w in range(NWIN):
                        for j in range(int(cbr[b, w])):
                            cg = int(cell_off[w, b]) + j       # global chunk
                            cw = cg - int(woff[w])             # within stream
                            s = cw // SPANC
                            ensure_span(w, s)
                            gt = span_tiles[w][s]
                            co = cw - s * SPANC
                            s_t = spool.tile([P, P], f16, tag="s")
                            nc.vector.tensor_scalar(
                                s_t[:], iota_sb[:],
                                dstl_sb[:, cg:cg + 1],
                                wgt_sb[:, cg:cg + 1],
                                ALU.is_equal, ALU.mult)
                            nc.tensor.matmul(
                                ps_agg[:],
                                lhsT=gt[:, co * gw: co * gw + fa],
                                rhs=s_t[:],
                                start=(done == 0), stop=(done == total - 1))
                            done += 1
                    # epilogue
                    if li == 0:
                        # psum = (sum_e w_e feats[src_e])^T [feat, dst]
                        mt_sb = mpool.tile([P, P], f32, tag="mt")
                        nc.scalar.activation(mt_sb[:], ps_agg[:], AF.Copy)
                        ps_h = pspool.tile([F_HID, P], f32, tag="ph")
                        nc.tensor.matmul(ps_h[:], lhsT=w_sb[0][:],
                                         rhs=mt_sb[:], start=True, stop=True)
                        h_sb = mpool.tile([F_HID, P], f32, tag="h")
                        nc.scalar.activation(h_sb[:], ps_h[:], AF.Relu,
                                             bias=b_sb[0][:])
                        ps_t = pspool.tile([F_HID, P], f32, tag="pt")
                        nc.tensor.matmul(ps_t[:], lhsT=w_sb[1][:],
                                         rhs=h_sb[:], start=True, stop=True)
                        tt_sb = mpool.tile([F_HID, P], f16, tag="tt")
                        nc.scalar.activation(tt_sb[:], ps_t[:], AF.Copy)
                        ps_tr = pspool.tile([P, F_HID], f16, tag="ptr")
                        nc.tensor.transpose(ps_tr[:], tt_sb[:], ident_sb[:])
                        y_sb = mpool.tile([P, F_HID], f16, tag="y")
                        nc.vector.tensor_copy(y_sb[:], ps_tr[:])
                        nc.sync.dma_start(
                            out=ag1[b * P:(b + 1) * P, :], in_=y_sb[:])
                    elif li == 1:
                        # psum = (agg*nd)^T [f1', dst]; produce t2 block
                        h_sb = mpool.tile([F_HID, P], f32, tag="h")
                        nc.scalar.activation(h_sb[:], ps_agg[:], AF.Relu,
                                             bias=b_sb[1][:])
                        ps_t = pspool.tile([F_OUT, P], f32, tag="pt")
                        nc.tensor.matmul(ps_t[:], lhsT=w_sb[2][:],
                                         rhs=h_sb[:], start=True, stop=True)
                        tt_sb = mpool.tile([F_OUT, P], f16, tag="tt")
                        nc.scalar.activation(tt_sb[:], ps_t[:], AF.Copy)
                        ps_tr = pspool.tile([P, F_OUT], f16, tag="ptr")
                        nc.tensor.transpose(ps_tr[:], tt_sb[:],
                                            ident_sb[:F_OUT, :F_OUT])
                        y_sb = mpool.tile([P, F_OUT], f16, tag="y")
                        nc.vector.tensor_copy(y_sb[:], ps_tr[:])
                        nc.sync.dma_start(
                            out=ag2[b * P:(b + 1) * P, :F_OUT], in_=y_sb[:])
                    else:
                        # psum = (agg*nd)^T [64, dst]; final output
                        yt_sb = mpool.tile([F_OUT, P], f32, tag="yt")
                        nc.scalar.activation(yt_sb[:], ps_agg[:], AF.Identity,
                                             bias=b_sb[2][:])
                        ps_tr = pspool.tile([P, F_OUT], f32, tag="ptr")
                        nc.tensor.transpose(ps_tr[:], yt_sb[:],
                                            identf_sb[:F_OUT, :F_OUT])
                        y_sb = mpool.tile([P, F_OUT], f32, tag="y")
                        nc.vector.tensor_copy(y_sb[:], ps_tr[:])
                        hi = min((b + 1) * P, SHARD)
                        nc.sync.dma_start(out=out_d[b * P:hi, :],
                                          in_=y_sb[:hi - b * P, :])
                    # split AllGather: first half as soon as block 48 done
                    if li < 2 and b == BLOCKS // 2 - 1:
                        ag_t = ag1 if li == 0 else ag2
                        tab = t1_ta if li == 0 else t2_ta
                        nc.gpsimd.collective_compute(
                            "AllGather", mybir.AluOpType.bypass,
                            replica_groups=[list(range(NCORES))],
                            ins=[ag_t[0:HALF, :].opt()],
                            outs=[tab[:].opt()],
                        )
                if li < 2:
                    ag_t = ag1 if li == 0 else ag2
                    tab = t1_tb if li == 0 else t2_tb
                    nc.gpsimd.collective_compute(
                        "AllGather", mybir.AluOpType.bypass,
                        replica_groups=[list(range(NCORES))],
                        ins=[ag_t[HALF:SHARD_PAD, :].opt()],
                        outs=[tab[:].opt()],
                    )

            def _tslice(ta, tb):
                def f(w):
                    if WBOUND[w] < HALFTAB:
                        return ta[WBOUND[w]:WBOUND[w + 1], :]
                    return tb[WBOUND[w] - HALFTAB:WBOUND[w + 1] - HALFTAB, :]
                return f

            layer(0, lambda w: feats[WBOUND[w]:WBOUND[w + 1], :], F_IN, True)
            layer(1, _tslice(t1_ta, t1_tb), F_HID, True)
            layer(2, _tslice(t2_ta, t2_tb), P, False)

    nc.compile()
    return nc


def kernel(**inputs):
    global LAST_EXEC_NS, LAST_RESULTS
    from concourse.bass_utils import run_bass_kernel_spmd

    cbr, gidx, dstl, wgt = _preprocess(inputs["src"], inputs["dst"])
    nc = _build_program(cbr)

    feats = np.asarray(inputs["features"], dtype=np.float32)
    featpad = np.zeros((TAB_PAD, F_IN), np.float16)
    rows = _tab_row(np.arange(N_NODES))
    featpad[rows] = feats.astype(np.float16)

    common = {
        "featpad": featpad,
        "W0": np.asarray(inputs["W0"], dtype=np.float32),
        "W1": np.asarray(inputs["W1"], dtype=np.float32),
        "W2": np.asarray(inputs["W2"], dtype=np.float32),
        "b0": np.asarray(inputs["b0"], dtype=np.float32),
        "b1": np.asarray(inputs["b1"], dtype=np.float32),
        "b2": np.asarray(inputs["b2"], dtype=np.float32),
        "iota": np.tile(np.arange(P, dtype=np.float16), (P, 1)),
        "identity": np.eye(P, dtype=np.float16),
        "identityf": np.eye(P, dtype=np.float32),
    }
    in_maps = []
    for c in range(NCORES):
        m = dict(common)
        m["gidx"] = gidx[c]
        m["dstl"] = dstl[c]
        m["wgt"] = wgt[c]
        in_maps.append(m)

    trace = bool(int(os.environ.get("BASS_GNN_TRACE", "0")))
    kwargs = {}
    if trace:
        _register_ntff_hook()
        kwargs = dict(trace=True,
                      tmpdir=os.environ.get("BASS_GNN_TRACE_DIR") or None)
    res = run_bass_kernel_spmd(nc, in_maps, core_ids=list(range(NCORES)),
                               **kwargs)
    LAST_EXEC_NS = res.exec_time_ns
    LAST_RESULTS = res
    out = np.concatenate([res.results[c]["out"] for c in range(NCORES)],
                         axis=0)
    return np.ascontiguousarray(out.astype(np.float32))


def _register_ntff_hook():
    """The container's antenv lacks axon_hooks; register the NTFF profile
    hook ourselves so trace=True works under axon."""
    import sys, types
    if "antenv.axon_hooks" in sys.modules:
        return
    try:
        import antenv
        from trn_agent_boot.trn_boot import _ntff_profile_via_ctypes
        mod = types.ModuleType("antenv.axon_hooks")
        mod._hook = _ntff_profile_via_ctypes('/opt/axon/libaxon_pjrt.so')
        mod.set_axon_ntff_profile_hook = lambda h: setattr(mod, "_hook", h)
        mod.get_axon_ntff_profile_hook = lambda: mod._hook
        sys.modules["antenv.axon_hooks"] = mod
        antenv.axon_hooks = mod
    except Exception as e:
        print("ntff hook registration failed:", e)
